# revision 5
# baseline (speedup 1.0000x reference)
"""HGNN conv distributed Bass kernel for 8 TRN2 NeuronCores.

Computes  out = 0.5 * D_e ⊙ (MT.T @ (D_v ⊙ (MT @ (x @ W))))
with N=16384 nodes, E=8192 hyperedges, IN_FT=256, OUT_FT=128.

Sharding (node/data parallel per hint): MT columns, x rows and D_e are
sharded over nodes across the 8 cores; W and D_v are replicated. The
MT @ y contraction over nodes becomes a partial sum + AllReduce; the
MT.T @ z contraction over edges is local per node shard.

Per core the MT shard [E, N/8] streams through SBUF exactly once
(bf16, host-cast), fused over both phases in superblocks of EB edges:
  phase 1 needs MT.T tiles (contraction over nodes -> nodes on
  partitions), produced by PE-transposes; eyT partials accumulate in
  PSUM, then AllReduce across cores (pipelined, 2 superblocks of
  slack so the collective hides behind PE work).
  phase 2 uses the natural MT tiles with the reduced z as stationary,
  accumulating nyT in 4 persistent PSUM banks across all superblocks.
"""

import functools
from contextlib import ExitStack

import ml_dtypes
import numpy as np

import concourse.bass as bass
import concourse.mybir as mybir
import concourse.tile as tile
from concourse import bacc
from concourse.bass_utils import run_bass_kernel_spmd
from concourse.masks import make_identity

P = 128
BF16 = mybir.dt.bfloat16
F32 = mybir.dt.float32

FULL_CFG = dict(N=16384, E=8192, IN=256, F=128, CORES=8, G=8)


def _ceil_div(a, b):
    return -(-a // b)


def build_kernel(nc, cfg):
    N, E, IN, F, CORES, G = (
        cfg["N"], cfg["E"], cfg["IN"], cfg["F"], cfg["CORES"], cfg["G"])
    NS = N // CORES          # nodes per core
    EB = E // G              # edges per superblock
    ET = EB // P             # 128-edge chunks per superblock
    NJ = NS // P             # 128-node chunks (phase-1 contraction)
    KI = IN // P             # 128-in_ft chunks
    EH = _ceil_div(EB, 512)  # 512-edge groups per superblock (phase-1 psum)
    NQ = _ceil_div(NS, 512)  # 512-node groups (phase-2 free dim)
    EW = min(EB, 512)        # phase-1 psum group width
    NW = min(NS, 512)        # phase-2 moving free width
    assert EB % P == 0 and NS % P == 0 and IN % P == 0 and F == P

    mt = nc.dram_tensor("mt", [E, NS], BF16, kind="ExternalInput").ap()
    xs = nc.dram_tensor("xs", [NS, IN], BF16, kind="ExternalInput").ap()
    w = nc.dram_tensor("w", [IN, F], BF16, kind="ExternalInput").ap()
    dvt = nc.dram_tensor("dvt", [P, E // P], F32, kind="ExternalInput").ap()
    det = nc.dram_tensor("det", [P, NJ], F32, kind="ExternalInput").ap()
    out = nc.dram_tensor("out", [NS, F], F32, kind="ExternalOutput").ap()

    with tile.TileContext(nc) as tc, ExitStack() as ctx:
        consts = ctx.enter_context(tc.tile_pool(name="consts", bufs=1))
        sbig = ctx.enter_context(tc.tile_pool(name="sbig", bufs=1))
        mtp = ctx.enter_context(tc.tile_pool(name="mtp", bufs=3))
        mtT_p = ctx.enter_context(tc.tile_pool(name="mtT", bufs=4))
        eyp_p = ctx.enter_context(tc.tile_pool(name="eyp", bufs=2))
        eyf_p = ctx.enter_context(tc.tile_pool(name="eyf", bufs=3))
        z_p = ctx.enter_context(tc.tile_pool(name="zp", bufs=3))
        ps_tr = ctx.enter_context(tc.tile_pool(name="ps_tr", bufs=2, space="PSUM"))
        ps_ey = ctx.enter_context(tc.tile_pool(name="ps_ey", bufs=1, space="PSUM"))
        ps_ny = ctx.enter_context(tc.tile_pool(name="ps_ny", bufs=1, space="PSUM"))
        dram = ctx.enter_context(tc.tile_pool(name="dram", bufs=2, space="DRAM"))

        id16 = consts.tile([P, P], BF16, tag="id16")
        id32 = consts.tile([P, P], F32, tag="id32")
        make_identity(nc, id16[:])
        make_identity(nc, id32[:])

        w_sb = consts.tile([P, KI, F], BF16, tag="w")
        nc.sync.dma_start(w_sb[:], w.rearrange("(k p) f -> p k f", p=P))
        dvt_sb = consts.tile([P, E // P], F32, tag="dvt")
        nc.sync.dma_start(dvt_sb[:], dvt)
        det_sb = consts.tile([P, NJ], F32, tag="det")
        nc.sync.dma_start(det_sb[:], det)

        xs_sb = sbig.tile([P, NJ, IN], BF16, tag="xs")
        nc.sync.dma_start(xs_sb[:], xs.rearrange("(i p) c -> p i c", p=P))
        xsT_sb = sbig.tile([P, KI, NS], BF16, tag="xsT")
        y_sb = sbig.tile([P, NS], BF16, tag="y")

        # Copy-engine alternation between DVE and ACT to split PSUM->SBUF load.
        cp_state = [0]

        def copy_eng():
            cp_state[0] ^= 1
            if cp_state[0]:
                return nc.vector.tensor_copy
            return nc.scalar.copy

        # ---- Step A: y = xs @ w ------------------------------------------
        for k in range(KI):
            for i0 in range(0, NJ, 4):
                nch = min(4, NJ - i0)
                tr = ps_tr.tile([P, 512], BF16, tag="tr")
                for c in range(nch):
                    nc.tensor.transpose(
                        tr[:, c * P:(c + 1) * P],
                        xs_sb[:, i0 + c, k * P:(k + 1) * P],
                        id16[:],
                    )
                copy_eng()(
                    xsT_sb[:, k, i0 * P:(i0 + nch) * P], tr[:, : nch * P])
        for i in range(NJ):
            yp = ps_tr.tile([P, F], F32, tag="tr")
            for k in range(KI):
                nc.tensor.matmul(
                    yp[:],
                    lhsT=xsT_sb[:, k, i * P:(i + 1) * P],
                    rhs=w_sb[:, k, :],
                    start=(k == 0),
                    stop=(k == KI - 1),
                )
            nc.vector.tensor_copy(y_sb[:, i * P:(i + 1) * P], yp[:])

        nyT = ps_ny.tile([P, NS], F32, tag="ny")

        # ---- Main loop over superblocks ----------------------------------
        def emit_p1_block(g):
            mt_sb = mtp.tile([P, ET, NS], BF16, tag="mt")
            nc.sync.dma_start(
                mt_sb[:],
                mt[g * EB:(g + 1) * EB, :].rearrange("(t p) n -> p t n", p=P),
            )
            eyT = ps_ey.tile([P, EB], F32, tag="ey")
            for j in range(NJ):
                mtT = mtT_p.tile([P, EB], BF16, tag="mtT")
                for h in range(EH):
                    nch = min(4, ET - h * 4)
                    tr = ps_tr.tile([P, 512], BF16, tag="tr")
                    for c in range(nch):
                        t = h * 4 + c
                        nc.tensor.transpose(
                            tr[:, c * P:(c + 1) * P],
                            mt_sb[:, t, j * P:(j + 1) * P],
                            id16[:],
                        )
                    copy_eng()(
                        mtT[:, h * EW:h * EW + nch * P], tr[:, : nch * P])
                for h in range(EH):
                    hw = min(EW, EB - h * EW)
                    nc.tensor.matmul(
                        eyT[:, h * EW:h * EW + hw],
                        lhsT=y_sb[:, j * P:(j + 1) * P],
                        rhs=mtT[:, h * EW:h * EW + hw],
                        start=(j == 0),
                        stop=(j == NJ - 1),
                    )
            eyp = eyp_p.tile([P, EB], F32, tag="eyp")
            for h in range(EH):
                hw = min(EW, EB - h * EW)
                nc.vector.tensor_copy(
                    eyp[:, h * EW:h * EW + hw], eyT[:, h * EW:h * EW + hw])
            bin_t = dram.tile([P, EB], F32, tag="bin")
            bout_t = dram.tile([P, EB], F32, tag="bout")
            nc.sync.dma_start(bin_t[:], eyp[:])
            nc.gpsimd.collective_compute(
                "AllReduce",
                mybir.AluOpType.add,
                replica_groups=[list(range(CORES))],
                ins=[bin_t.opt()],
                outs=[bout_t.opt()],
            )
            eyf = eyf_p.tile([P, EB], BF16, tag="eyf")
            nc.gpsimd.dma_start(eyf[:], bout_t[:])  # f32 -> bf16 cast
            return mt_sb, eyf

        def emit_p2_block(g, mt_sb, eyf):
            z = z_p.tile([P, EB], BF16, tag="z")
            for h in range(EH):
                nch = min(4, ET - h * 4)
                tr = ps_tr.tile([P, 512], BF16, tag="tr")
                for c in range(nch):
                    t = h * 4 + c
                    nc.tensor.transpose(
                        tr[:, c * P:(c + 1) * P],
                        eyf[:, t * P:(t + 1) * P],
                        id16[:],
                    )
                for c in range(nch):
                    t = h * 4 + c
                    nc.vector.tensor_scalar_mul(
                        z[:, t * P:(t + 1) * P],
                        tr[:, c * P:(c + 1) * P],
                        dvt_sb[:, g * ET + t:g * ET + t + 1],
                    )
            for t in range(ET):
                for q in range(NQ):
                    qw = min(NW, NS - q * NW)
                    nc.tensor.matmul(
                        nyT[:, q * NW:q * NW + qw],
                        lhsT=z[:, t * P:(t + 1) * P],
                        rhs=mt_sb[:, t, q * NW:q * NW + qw],
                        start=(g == 0 and t == 0),
                        stop=(g == G - 1 and t == ET - 1),
                    )

        pending = []
        for g in range(G):
            pending.append(emit_p1_block(g))
            if g >= 2:
                emit_p2_block(g - 2, *pending[g - 2])
        for g in range(max(0, G - 2), G):
            emit_p2_block(g, *pending[g])

        # ---- Finalize: out = det ⊙ nyT.T ---------------------------------
        ny_sb = sbig.tile([P, NS], F32, tag="ny_sb")
        for q in range(NQ):
            qw = min(NW, NS - q * NW)
            nc.vector.tensor_copy(
                ny_sb[:, q * NW:q * NW + qw], nyT[:, q * NW:q * NW + qw])
        out_sb = sbig.tile([P, NS], F32, tag="out_sb")
        for i0 in range(0, NJ, 4):
            nch = min(4, NJ - i0)
            tr = ps_tr.tile([P, 512], F32, tag="tr")
            for c in range(nch):
                i = i0 + c
                nc.tensor.transpose(
                    tr[:, c * P:(c + 1) * P],
                    ny_sb[:, i * P:(i + 1) * P],
                    id32[:],
                )
            for c in range(nch):
                i = i0 + c
                nc.vector.tensor_scalar_mul(
                    out_sb[:, i * P:(i + 1) * P],
                    tr[:, c * P:(c + 1) * P],
                    det_sb[:, i:i + 1],
                )
        nc.sync.dma_start(
            out.rearrange("(i p) f -> p i f", p=P),
            out_sb[:].rearrange("p (i f) -> p i f", f=F))

    return nc


@functools.lru_cache(maxsize=2)
def _compiled(cfg_items):
    cfg = dict(cfg_items)
    nc = bacc.Bacc(
        "TRN2",
        target_bir_lowering=False,
        debug=False,
        num_devices=cfg["CORES"],
    )
    build_kernel(nc, cfg)
    nc.compile()
    return nc


def shard_inputs(x, weight, MT, D_v_diag, D_e_diag, cfg):
    """Host-side sharding + dtype prep. Returns in_maps for the 8 cores."""
    N, E, IN, F, CORES = cfg["N"], cfg["E"], cfg["IN"], cfg["F"], cfg["CORES"]
    NS = N // CORES
    bf = ml_dtypes.bfloat16
    w_b = np.ascontiguousarray(np.asarray(weight, dtype=np.float32)).astype(bf)
    x_f = np.asarray(x, dtype=np.float32)
    mt_f = np.asarray(MT, dtype=np.float32)
    dv = np.asarray(D_v_diag, dtype=np.float32)
    de = np.asarray(D_e_diag, dtype=np.float32)
    # [P, E/P] with chunk index on the free axis
    dvt = np.ascontiguousarray(dv.reshape(E // 128, 128).T)
    in_maps = []
    for c in range(CORES):
        sl = slice(c * NS, (c + 1) * NS)
        det = np.ascontiguousarray(
            (0.5 * de[sl]).reshape(NS // 128, 128).T)
        in_maps.append({
            "mt": np.ascontiguousarray(mt_f[:, sl]).astype(bf),
            "xs": np.ascontiguousarray(x_f[sl]).astype(bf),
            "w": w_b,
            "dvt": dvt,
            "det": det,
        })
    return in_maps


def _run(x, weight, MT, D_v_diag, D_e_diag, cfg=None, trace=False):
    cfg = cfg or FULL_CFG
    nc = _compiled(tuple(sorted(cfg.items())))
    in_maps = shard_inputs(x, weight, MT, D_v_diag, D_e_diag, cfg)
    res = run_bass_kernel_spmd(
        nc, in_maps, core_ids=list(range(cfg["CORES"])), trace=trace)
    NS = cfg["N"] // cfg["CORES"]
    out = np.concatenate(
        [np.asarray(res.results[c]["out"]) for c in range(cfg["CORES"])],
        axis=0,
    ).astype(np.float32)
    return out, res


def kernel(x, weight, MT, D_v_diag, D_e_diag):
    out, _ = _run(x, weight, MT, D_v_diag, D_e_diag)
    return out


# revision 6
# speedup vs baseline: 1.2778x; 1.2778x over previous
"""HGNN conv distributed Bass kernel for 8 TRN2 NeuronCores.

Computes  out = 0.5 * D_e ⊙ (MT.T @ (D_v ⊙ (MT @ (x @ W))))
with N=16384 nodes, E=8192 hyperedges, IN_FT=256, OUT_FT=128.

Sharding (node/data parallel per hint): MT columns, x rows and D_e are
sharded over nodes across the 8 cores; W and D_v are replicated. The
MT @ y contraction over nodes becomes a partial sum + AllReduce; the
MT.T @ z contraction over edges is local per node shard.

Per core the MT shard [E, N/8] streams through SBUF exactly once
(bf16, host-cast), fused over both phases in superblocks of EB edges:
  phase 1 needs MT.T tiles (contraction over nodes -> nodes on
  partitions), produced by PE-transposes; eyT partials accumulate in
  PSUM, then AllReduce across cores (pipelined, 2 superblocks of
  slack so the collective hides behind PE work).
  phase 2 uses the natural MT tiles with the reduced z as stationary,
  accumulating nyT in 4 persistent PSUM banks across all superblocks.
"""

import functools
from contextlib import ExitStack

import ml_dtypes
import numpy as np

import concourse.bass as bass
import concourse.mybir as mybir
import concourse.tile as tile
from concourse import bacc
from concourse.bass_utils import run_bass_kernel_spmd
from concourse.masks import make_identity

P = 128
BF16 = mybir.dt.bfloat16
F32 = mybir.dt.float32

FULL_CFG = dict(N=16384, E=8192, IN=256, F=128, CORES=8, G=8)


def _ceil_div(a, b):
    return -(-a // b)


def build_kernel(nc, cfg):
    N, E, IN, F, CORES, G = (
        cfg["N"], cfg["E"], cfg["IN"], cfg["F"], cfg["CORES"], cfg["G"])
    NS = N // CORES          # nodes per core
    EB = E // G              # edges per superblock
    ET = EB // P             # 128-edge chunks per superblock
    NJ = NS // P             # 128-node chunks (phase-1 contraction)
    KI = IN // P             # 128-in_ft chunks
    EH = _ceil_div(EB, 512)  # 512-edge groups per superblock (phase-1 psum)
    NQ = _ceil_div(NS, 512)  # 512-node groups (phase-2 free dim)
    EW = min(EB, 512)        # phase-1 psum group width
    NW = min(NS, 512)        # phase-2 moving free width
    assert EB % P == 0 and NS % P == 0 and IN % P == 0 and F == P

    mt = nc.dram_tensor("mt", [E, NS], BF16, kind="ExternalInput").ap()
    xs = nc.dram_tensor("xs", [NS, IN], BF16, kind="ExternalInput").ap()
    w = nc.dram_tensor("w", [IN, F], BF16, kind="ExternalInput").ap()
    dvt = nc.dram_tensor("dvt", [P, E // P], F32, kind="ExternalInput").ap()
    det = nc.dram_tensor("det", [P, NJ], F32, kind="ExternalInput").ap()
    out = nc.dram_tensor("out", [NS, F], F32, kind="ExternalOutput").ap()

    with tile.TileContext(nc) as tc, ExitStack() as ctx:
        consts = ctx.enter_context(tc.tile_pool(name="consts", bufs=1))
        sbig = ctx.enter_context(tc.tile_pool(name="sbig", bufs=1))
        mtp = ctx.enter_context(tc.tile_pool(name="mtp", bufs=4))
        mtT_p = ctx.enter_context(tc.tile_pool(name="mtT", bufs=4))
        xsT_p = ctx.enter_context(tc.tile_pool(name="xsT_p", bufs=3))
        eyp_p = ctx.enter_context(tc.tile_pool(name="eyp", bufs=2))
        eyf_p = ctx.enter_context(tc.tile_pool(name="eyf", bufs=4))
        z_p = ctx.enter_context(tc.tile_pool(name="zp", bufs=3))
        ps_tr = ctx.enter_context(tc.tile_pool(name="ps_tr", bufs=4, space="PSUM"))
        ps_ey = ctx.enter_context(tc.tile_pool(name="ps_ey", bufs=1, space="PSUM"))
        ps_p2 = ctx.enter_context(tc.tile_pool(name="ps_p2", bufs=2, space="PSUM"))
        dram = ctx.enter_context(tc.tile_pool(name="dram", bufs=2, space="DRAM"))

        id16 = consts.tile([P, P], BF16, tag="id16")
        id32 = consts.tile([P, P], F32, tag="id32")
        make_identity(nc, id16[:])
        make_identity(nc, id32[:])

        w_sb = consts.tile([P, KI, F], BF16, tag="w")
        nc.sync.dma_start(w_sb[:], w.rearrange("(k p) f -> p k f", p=P))
        dvt_sb = consts.tile([P, E // P], F32, tag="dvt")
        nc.sync.dma_start(dvt_sb[:], dvt)
        det_sb = consts.tile([P, NJ], F32, tag="det")
        nc.sync.dma_start(det_sb[:], det)

        xs_sb = sbig.tile([P, NJ, IN], BF16, tag="xs")
        nc.sync.dma_start(xs_sb[:], xs.rearrange("(i p) c -> p i c", p=P))
        y_sb = sbig.tile([P, NS], BF16, tag="y")
        ny_sb = sbig.tile([P, NS], F32, tag="ny_sb")

        # Copy-engine alternation between DVE and ACT to split PSUM->SBUF load.
        cp_state = [0]

        def copy_eng():
            cp_state[0] ^= 1
            if cp_state[0]:
                return nc.vector.tensor_copy
            return nc.scalar.copy

        # ---- Step A: y = xs @ w (software-pipelined by one i) ------------
        def a_transpose(i):
            tr = ps_tr.tile([P, 512], BF16, tag="tr")
            xsT = xsT_p.tile([P, KI * P], BF16, tag="xsT")
            for k in range(KI):
                nc.tensor.transpose(
                    tr[:, k * P:(k + 1) * P],
                    xs_sb[:, i, k * P:(k + 1) * P],
                    id16[:],
                )
            copy_eng()(xsT[:], tr[:, : KI * P])
            return xsT

        def a_matmul(i, xsT):
            yp = ps_tr.tile([P, F], F32, tag="tr")
            for k in range(KI):
                nc.tensor.matmul(
                    yp[:],
                    lhsT=xsT[:, k * P:(k + 1) * P],
                    rhs=w_sb[:, k, :],
                    start=(k == 0),
                    stop=(k == KI - 1),
                )
            nc.vector.tensor_copy(y_sb[:, i * P:(i + 1) * P], yp[:])

        xsT_prev = None
        for i in range(NJ):
            xsT_cur = a_transpose(i)
            if xsT_prev is not None:
                a_matmul(i - 1, xsT_prev)
            xsT_prev = xsT_cur
        a_matmul(NJ - 1, xsT_prev)

        # ---- Main loop over superblocks ----------------------------------
        def emit_p1_block(g):
            mt_sb = mtp.tile([P, ET, NS], BF16, tag="mt")
            nc.sync.dma_start(
                mt_sb[:],
                mt[g * EB:(g + 1) * EB, :].rearrange("(t p) n -> p t n", p=P),
            )
            eyT = ps_ey.tile([P, EB], F32, tag="ey")

            def p1_transpose(j):
                mtT = mtT_p.tile([P, EB], BF16, tag="mtT")
                for h in range(EH):
                    nch = min(4, ET - h * 4)
                    tr = ps_tr.tile([P, 512], BF16, tag="tr")
                    for c in range(nch):
                        t = h * 4 + c
                        nc.tensor.transpose(
                            tr[:, c * P:(c + 1) * P],
                            mt_sb[:, t, j * P:(j + 1) * P],
                            id16[:],
                        )
                    copy_eng()(
                        mtT[:, h * EW:h * EW + nch * P], tr[:, : nch * P])
                return mtT

            def p1_matmul(j, mtT):
                for h in range(EH):
                    hw = min(EW, EB - h * EW)
                    nc.tensor.matmul(
                        eyT[:, h * EW:h * EW + hw],
                        lhsT=y_sb[:, j * P:(j + 1) * P],
                        rhs=mtT[:, h * EW:h * EW + hw],
                        start=(j == 0),
                        stop=(j == NJ - 1),
                    )

            mtT_prev = None
            for j in range(NJ):
                mtT_cur = p1_transpose(j)
                if mtT_prev is not None:
                    p1_matmul(j - 1, mtT_prev)
                mtT_prev = mtT_cur
            p1_matmul(NJ - 1, mtT_prev)

            eyp = eyp_p.tile([P, EB], BF16, tag="eyp")
            for h in range(EH):
                hw = min(EW, EB - h * EW)
                nc.vector.tensor_copy(
                    eyp[:, h * EW:h * EW + hw], eyT[:, h * EW:h * EW + hw])
            bin_t = dram.tile([P, EB], BF16, tag="bin")
            bout_t = dram.tile([P, EB], BF16, tag="bout")
            nc.sync.dma_start(bin_t[:], eyp[:])
            nc.gpsimd.collective_compute(
                "AllReduce",
                mybir.AluOpType.add,
                replica_groups=[list(range(CORES))],
                ins=[bin_t.opt()],
                outs=[bout_t.opt()],
            )
            eyf = eyf_p.tile([P, EB], BF16, tag="eyf")
            nc.sync.dma_start(eyf[:], bout_t[:])
            return mt_sb, eyf

        def emit_p2_block(g, mt_sb, eyf):
            z = z_p.tile([P, EB], BF16, tag="z")
            for h in range(EH):
                nch = min(4, ET - h * 4)
                tr = ps_tr.tile([P, 512], BF16, tag="tr")
                for c in range(nch):
                    t = h * 4 + c
                    nc.tensor.transpose(
                        tr[:, c * P:(c + 1) * P],
                        eyf[:, t * P:(t + 1) * P],
                        id16[:],
                    )
                for c in range(nch):
                    t = h * 4 + c
                    nc.vector.tensor_scalar_mul(
                        z[:, t * P:(t + 1) * P],
                        tr[:, c * P:(c + 1) * P],
                        dvt_sb[:, g * ET + t:g * ET + t + 1],
                    )
            for q in range(NQ):
                qw = min(NW, NS - q * NW)
                p2 = ps_p2.tile([P, NW], F32, tag="p2")
                for t in range(ET):
                    nc.tensor.matmul(
                        p2[:, :qw],
                        lhsT=z[:, t * P:(t + 1) * P],
                        rhs=mt_sb[:, t, q * NW:q * NW + qw],
                        start=(t == 0),
                        stop=(t == ET - 1),
                    )
                if g == 0:
                    nc.vector.tensor_copy(
                        ny_sb[:, q * NW:q * NW + qw], p2[:, :qw])
                else:
                    nc.vector.tensor_add(
                        ny_sb[:, q * NW:q * NW + qw],
                        ny_sb[:, q * NW:q * NW + qw],
                        p2[:, :qw],
                    )

        SLACK = 3
        pending = []
        for g in range(G):
            pending.append(emit_p1_block(g))
            if g >= SLACK:
                emit_p2_block(g - SLACK, *pending[g - SLACK])
        for g in range(max(0, G - SLACK), G):
            emit_p2_block(g, *pending[g])

        # ---- Finalize: out = det ⊙ ny_sb.T -------------------------------
        out_sb = sbig.tile([P, NS], F32, tag="out_sb")
        for i0 in range(0, NJ, 4):
            nch = min(4, NJ - i0)
            tr = ps_tr.tile([P, 512], F32, tag="tr")
            for c in range(nch):
                i = i0 + c
                nc.tensor.transpose(
                    tr[:, c * P:(c + 1) * P],
                    ny_sb[:, i * P:(i + 1) * P],
                    id32[:],
                )
            for c in range(nch):
                i = i0 + c
                nc.vector.tensor_scalar_mul(
                    out_sb[:, i * P:(i + 1) * P],
                    tr[:, c * P:(c + 1) * P],
                    det_sb[:, i:i + 1],
                )
        nc.sync.dma_start(
            out.rearrange("(i p) f -> p i f", p=P),
            out_sb[:].rearrange("p (i f) -> p i f", f=F))

    return nc


@functools.lru_cache(maxsize=2)
def _compiled(cfg_items):
    cfg = dict(cfg_items)
    nc = bacc.Bacc(
        "TRN2",
        target_bir_lowering=False,
        debug=False,
        num_devices=cfg["CORES"],
    )
    build_kernel(nc, cfg)
    nc.compile()
    return nc


def shard_inputs(x, weight, MT, D_v_diag, D_e_diag, cfg):
    """Host-side sharding + dtype prep. Returns in_maps for the 8 cores."""
    N, E, IN, F, CORES = cfg["N"], cfg["E"], cfg["IN"], cfg["F"], cfg["CORES"]
    NS = N // CORES
    bf = ml_dtypes.bfloat16
    w_b = np.ascontiguousarray(np.asarray(weight, dtype=np.float32)).astype(bf)
    x_f = np.asarray(x, dtype=np.float32)
    mt_f = np.asarray(MT, dtype=np.float32)
    dv = np.asarray(D_v_diag, dtype=np.float32)
    de = np.asarray(D_e_diag, dtype=np.float32)
    # [P, E/P] with chunk index on the free axis
    dvt = np.ascontiguousarray(dv.reshape(E // 128, 128).T)
    in_maps = []
    for c in range(CORES):
        sl = slice(c * NS, (c + 1) * NS)
        det = np.ascontiguousarray(
            (0.5 * de[sl]).reshape(NS // 128, 128).T)
        in_maps.append({
            "mt": np.ascontiguousarray(mt_f[:, sl]).astype(bf),
            "xs": np.ascontiguousarray(x_f[sl]).astype(bf),
            "w": w_b,
            "dvt": dvt,
            "det": det,
        })
    return in_maps


def _run(x, weight, MT, D_v_diag, D_e_diag, cfg=None, trace=False):
    cfg = cfg or FULL_CFG
    nc = _compiled(tuple(sorted(cfg.items())))
    in_maps = shard_inputs(x, weight, MT, D_v_diag, D_e_diag, cfg)
    res = run_bass_kernel_spmd(
        nc, in_maps, core_ids=list(range(cfg["CORES"])), trace=trace)
    NS = cfg["N"] // cfg["CORES"]
    out = np.concatenate(
        [np.asarray(res.results[c]["out"]) for c in range(cfg["CORES"])],
        axis=0,
    ).astype(np.float32)
    return out, res


def kernel(x, weight, MT, D_v_diag, D_e_diag):
    out, _ = _run(x, weight, MT, D_v_diag, D_e_diag)
    return out


# revision 7
# speedup vs baseline: 1.3338x; 1.0438x over previous
"""HGNN conv distributed Bass kernel for 8 TRN2 NeuronCores.

Computes  out = 0.5 * D_e ⊙ (MT.T @ (D_v ⊙ (MT @ (x @ W))))
with N=16384 nodes, E=8192 hyperedges, IN_FT=256, OUT_FT=128.

Sharding (node/data parallel per hint): MT columns, x rows and D_e are
sharded over nodes across the 8 cores; W and D_v are replicated. The
MT @ y contraction over nodes becomes a partial sum + AllReduce; the
MT.T @ z contraction over edges is local per node shard.

Per core the MT shard [E, N/8] streams through SBUF exactly once
(bf16, host-cast), fused over both phases in superblocks of EB edges:
  phase 1 needs MT.T tiles (contraction over nodes -> nodes on
  partitions), produced by PE-transposes; eyT partials accumulate in
  PSUM, then AllReduce across cores (pipelined, 2 superblocks of
  slack so the collective hides behind PE work).
  phase 2 uses the natural MT tiles with the reduced z as stationary,
  accumulating nyT in 4 persistent PSUM banks across all superblocks.
"""

import functools
from contextlib import ExitStack

import ml_dtypes
import numpy as np

import concourse.bass as bass
import concourse.mybir as mybir
import concourse.tile as tile
from concourse import bacc
from concourse.bass_utils import run_bass_kernel_spmd
from concourse.masks import make_identity

P = 128
BF16 = mybir.dt.bfloat16
F32 = mybir.dt.float32

FULL_CFG = dict(N=16384, E=8192, IN=256, F=128, CORES=8, G=8)


def _ceil_div(a, b):
    return -(-a // b)


def build_kernel(nc, cfg):
    N, E, IN, F, CORES, G = (
        cfg["N"], cfg["E"], cfg["IN"], cfg["F"], cfg["CORES"], cfg["G"])
    NS = N // CORES          # nodes per core
    EB = E // G              # edges per superblock
    ET = EB // P             # 128-edge chunks per superblock
    NJ = NS // P             # 128-node chunks (phase-1 contraction)
    KI = IN // P             # 128-in_ft chunks
    EH = _ceil_div(EB, 512)  # 512-edge groups per superblock (phase-1 psum)
    NQ = _ceil_div(NS, 512)  # 512-node groups (phase-2 free dim)
    EW = min(EB, 512)        # phase-1 psum group width
    NW = min(NS, 512)        # phase-2 moving free width
    assert EB % P == 0 and NS % P == 0 and IN % P == 0 and F == P

    mt = nc.dram_tensor("mt", [E, NS], BF16, kind="ExternalInput").ap()
    xs = nc.dram_tensor("xs", [NS, IN], BF16, kind="ExternalInput").ap()
    w = nc.dram_tensor("w", [IN, F], BF16, kind="ExternalInput").ap()
    dvt = nc.dram_tensor("dvt", [P, E // P], F32, kind="ExternalInput").ap()
    det = nc.dram_tensor("det", [P, NJ], F32, kind="ExternalInput").ap()
    out = nc.dram_tensor("out", [NS, F], F32, kind="ExternalOutput").ap()

    with tile.TileContext(nc) as tc, ExitStack() as ctx:
        consts = ctx.enter_context(tc.tile_pool(name="consts", bufs=1))
        sbig = ctx.enter_context(tc.tile_pool(name="sbig", bufs=1))
        mtp = ctx.enter_context(tc.tile_pool(name="mtp", bufs=4))
        mtT_p = ctx.enter_context(tc.tile_pool(name="mtT", bufs=4))
        xsT_p = ctx.enter_context(tc.tile_pool(name="xsT_p", bufs=3))
        eyp_p = ctx.enter_context(tc.tile_pool(name="eyp", bufs=2))
        eyf_p = ctx.enter_context(tc.tile_pool(name="eyf", bufs=2))
        z_p = ctx.enter_context(tc.tile_pool(name="zp", bufs=3))
        ps_tr = ctx.enter_context(tc.tile_pool(name="ps_tr", bufs=4, space="PSUM"))
        ps_ey = ctx.enter_context(tc.tile_pool(name="ps_ey", bufs=1, space="PSUM"))
        ps_p2 = ctx.enter_context(tc.tile_pool(name="ps_p2", bufs=2, space="PSUM"))
        dram = ctx.enter_context(tc.tile_pool(name="dram", bufs=2, space="DRAM"))

        id16 = consts.tile([P, P], BF16, tag="id16")
        id32 = consts.tile([P, P], F32, tag="id32")
        make_identity(nc, id16[:])
        make_identity(nc, id32[:])

        w_sb = consts.tile([P, KI, F], BF16, tag="w")
        nc.sync.dma_start(w_sb[:], w.rearrange("(k p) f -> p k f", p=P))
        dvt_sb = consts.tile([P, E // P], F32, tag="dvt")
        nc.sync.dma_start(dvt_sb[:], dvt)
        det_sb = consts.tile([P, NJ], F32, tag="det")
        nc.sync.dma_start(det_sb[:], det)

        xs_sb = sbig.tile([P, NJ, IN], BF16, tag="xs")
        nc.sync.dma_start(xs_sb[:], xs.rearrange("(i p) c -> p i c", p=P))
        y_sb = sbig.tile([P, NS], BF16, tag="y")
        ny_sb = sbig.tile([P, NS], F32, tag="ny_sb")

        # Copy-engine alternation between DVE and ACT to split PSUM->SBUF load.
        cp_state = [0]

        def copy_eng():
            cp_state[0] ^= 1
            if cp_state[0]:
                return nc.vector.tensor_copy
            return nc.scalar.copy

        # ---- Step A: y = xs @ w (software-pipelined by one i) ------------
        def a_transpose(i):
            tr = ps_tr.tile([P, 512], BF16, tag="tr")
            xsT = xsT_p.tile([P, KI * P], BF16, tag="xsT")
            for k in range(KI):
                nc.tensor.transpose(
                    tr[:, k * P:(k + 1) * P],
                    xs_sb[:, i, k * P:(k + 1) * P],
                    id16[:],
                )
            copy_eng()(xsT[:], tr[:, : KI * P])
            return xsT

        def a_matmul(i, xsT):
            yp = ps_tr.tile([P, F], F32, tag="tr")
            for k in range(KI):
                nc.tensor.matmul(
                    yp[:],
                    lhsT=xsT[:, k * P:(k + 1) * P],
                    rhs=w_sb[:, k, :],
                    start=(k == 0),
                    stop=(k == KI - 1),
                )
            nc.vector.tensor_copy(y_sb[:, i * P:(i + 1) * P], yp[:])

        xsT_prev = None
        for i in range(NJ):
            xsT_cur = a_transpose(i)
            if xsT_prev is not None:
                a_matmul(i - 1, xsT_prev)
            xsT_prev = xsT_cur
        a_matmul(NJ - 1, xsT_prev)

        # ---- Main loop over superblocks ----------------------------------
        def emit_p1_block(g):
            mt_sb = mtp.tile([P, ET, NS], BF16, tag="mt")
            nc.sync.dma_start(
                mt_sb[:],
                mt[g * EB:(g + 1) * EB, :].rearrange("(t p) n -> p t n", p=P),
            )
            eyT = ps_ey.tile([P, EB], F32, tag="ey")

            def p1_transpose(j):
                mtT = mtT_p.tile([P, EB], BF16, tag="mtT")
                for h in range(EH):
                    nch = min(4, ET - h * 4)
                    tr = ps_tr.tile([P, 512], BF16, tag="tr")
                    for c in range(nch):
                        t = h * 4 + c
                        nc.tensor.transpose(
                            tr[:, c * P:(c + 1) * P],
                            mt_sb[:, t, j * P:(j + 1) * P],
                            id16[:],
                        )
                    copy_eng()(
                        mtT[:, h * EW:h * EW + nch * P], tr[:, : nch * P])
                return mtT

            def p1_matmul(j, mtT):
                for h in range(EH):
                    hw = min(EW, EB - h * EW)
                    nc.tensor.matmul(
                        eyT[:, h * EW:h * EW + hw],
                        lhsT=y_sb[:, j * P:(j + 1) * P],
                        rhs=mtT[:, h * EW:h * EW + hw],
                        start=(j == 0),
                        stop=(j == NJ - 1),
                    )

            mtT_prev = None
            for j in range(NJ):
                mtT_cur = p1_transpose(j)
                if mtT_prev is not None:
                    p1_matmul(j - 1, mtT_prev)
                mtT_prev = mtT_cur
            p1_matmul(NJ - 1, mtT_prev)

            eyp = eyp_p.tile([P, EB], BF16, tag="eyp")
            for h in range(EH):
                hw = min(EW, EB - h * EW)
                nc.vector.tensor_copy(
                    eyp[:, h * EW:h * EW + hw], eyT[:, h * EW:h * EW + hw])
            return mt_sb, eyp

        def emit_p2_block(g, mt_sb, eyf, off):
            z = z_p.tile([P, EB], BF16, tag="z")
            for h in range(EH):
                nch = min(4, ET - h * 4)
                tr = ps_tr.tile([P, 512], BF16, tag="tr")
                for c in range(nch):
                    t = h * 4 + c
                    nc.tensor.transpose(
                        tr[:, c * P:(c + 1) * P],
                        eyf[:, off + t * P:off + (t + 1) * P],
                        id16[:],
                    )
                for c in range(nch):
                    t = h * 4 + c
                    nc.vector.tensor_scalar_mul(
                        z[:, t * P:(t + 1) * P],
                        tr[:, c * P:(c + 1) * P],
                        dvt_sb[:, g * ET + t:g * ET + t + 1],
                    )
            for q in range(NQ):
                qw = min(NW, NS - q * NW)
                p2 = ps_p2.tile([P, NW], F32, tag="p2")
                for t in range(ET):
                    nc.tensor.matmul(
                        p2[:, :qw],
                        lhsT=z[:, t * P:(t + 1) * P],
                        rhs=mt_sb[:, t, q * NW:q * NW + qw],
                        start=(t == 0),
                        stop=(t == ET - 1),
                    )
                if g == 0:
                    nc.vector.tensor_copy(
                        ny_sb[:, q * NW:q * NW + qw], p2[:, :qw])
                else:
                    nc.vector.tensor_add(
                        ny_sb[:, q * NW:q * NW + qw],
                        ny_sb[:, q * NW:q * NW + qw],
                        p2[:, :qw],
                    )

        assert G % 2 == 0
        PAIRS = G // 2

        def emit_ar_pair(eyp_a, eyp_b):
            bin_t = dram.tile([P, 2 * EB], BF16, tag="bin")
            bout_t = dram.tile([P, 2 * EB], BF16, tag="bout")
            nc.sync.dma_start(bin_t[:, :EB], eyp_a[:])
            nc.sync.dma_start(bin_t[:, EB:], eyp_b[:])
            nc.gpsimd.collective_compute(
                "AllReduce",
                mybir.AluOpType.add,
                replica_groups=[list(range(CORES))],
                ins=[bin_t.opt()],
                outs=[bout_t.opt()],
            )
            eyf = eyf_p.tile([P, 2 * EB], BF16, tag="eyf")
            nc.sync.dma_start(eyf[:], bout_t[:])
            return eyf

        mts = []
        eyfs = {}

        def emit_p2_pair(p):
            emit_p2_block(2 * p, mts[2 * p], eyfs[p], 0)
            emit_p2_block(2 * p + 1, mts[2 * p + 1], eyfs[p], EB)

        for p in range(PAIRS):
            mt_a, eyp_a = emit_p1_block(2 * p)
            mt_b, eyp_b = emit_p1_block(2 * p + 1)
            mts += [mt_a, mt_b]
            eyfs[p] = emit_ar_pair(eyp_a, eyp_b)
            if p >= 1:
                emit_p2_pair(p - 1)
        emit_p2_pair(PAIRS - 1)

        # ---- Finalize: out = det ⊙ ny_sb.T -------------------------------
        out_sb = sbig.tile([P, NS], F32, tag="out_sb")
        for i0 in range(0, NJ, 4):
            nch = min(4, NJ - i0)
            tr = ps_tr.tile([P, 512], F32, tag="tr")
            for c in range(nch):
                i = i0 + c
                nc.tensor.transpose(
                    tr[:, c * P:(c + 1) * P],
                    ny_sb[:, i * P:(i + 1) * P],
                    id32[:],
                )
            for c in range(nch):
                i = i0 + c
                nc.vector.tensor_scalar_mul(
                    out_sb[:, i * P:(i + 1) * P],
                    tr[:, c * P:(c + 1) * P],
                    det_sb[:, i:i + 1],
                )
        nc.sync.dma_start(
            out.rearrange("(i p) f -> p i f", p=P),
            out_sb[:].rearrange("p (i f) -> p i f", f=F))

    return nc


@functools.lru_cache(maxsize=2)
def _compiled(cfg_items):
    cfg = dict(cfg_items)
    nc = bacc.Bacc(
        "TRN2",
        target_bir_lowering=False,
        debug=False,
        num_devices=cfg["CORES"],
    )
    build_kernel(nc, cfg)
    nc.compile()
    return nc


def shard_inputs(x, weight, MT, D_v_diag, D_e_diag, cfg):
    """Host-side sharding + dtype prep. Returns in_maps for the 8 cores."""
    N, E, IN, F, CORES = cfg["N"], cfg["E"], cfg["IN"], cfg["F"], cfg["CORES"]
    NS = N // CORES
    bf = ml_dtypes.bfloat16
    w_b = np.ascontiguousarray(np.asarray(weight, dtype=np.float32)).astype(bf)
    x_f = np.asarray(x, dtype=np.float32)
    mt_f = np.asarray(MT, dtype=np.float32)
    dv = np.asarray(D_v_diag, dtype=np.float32)
    de = np.asarray(D_e_diag, dtype=np.float32)
    # [P, E/P] with chunk index on the free axis
    dvt = np.ascontiguousarray(dv.reshape(E // 128, 128).T)
    in_maps = []
    for c in range(CORES):
        sl = slice(c * NS, (c + 1) * NS)
        det = np.ascontiguousarray(
            (0.5 * de[sl]).reshape(NS // 128, 128).T)
        in_maps.append({
            "mt": np.ascontiguousarray(mt_f[:, sl]).astype(bf),
            "xs": np.ascontiguousarray(x_f[sl]).astype(bf),
            "w": w_b,
            "dvt": dvt,
            "det": det,
        })
    return in_maps


def _run(x, weight, MT, D_v_diag, D_e_diag, cfg=None, trace=False):
    cfg = cfg or FULL_CFG
    nc = _compiled(tuple(sorted(cfg.items())))
    in_maps = shard_inputs(x, weight, MT, D_v_diag, D_e_diag, cfg)
    res = run_bass_kernel_spmd(
        nc, in_maps, core_ids=list(range(cfg["CORES"])), trace=trace)
    NS = cfg["N"] // cfg["CORES"]
    out = np.concatenate(
        [np.asarray(res.results[c]["out"]) for c in range(cfg["CORES"])],
        axis=0,
    ).astype(np.float32)
    return out, res


def kernel(x, weight, MT, D_v_diag, D_e_diag):
    out, _ = _run(x, weight, MT, D_v_diag, D_e_diag)
    return out
